# revision 3
# baseline (speedup 1.0000x reference)
"""Cepstrum -> minimum-phase impulse response on 8 Trainium2 NeuronCores.

Math: the reference recurrence  n*h_n = sum_k (k c_k) h_{n-k}, h_0 = exp(c_0)
is exactly the power-series exponential h = exp(C(z)) mod z^512 for the
degree-255 polynomial C. We evaluate it spectrally:

    h = IDFT_L( exp( DFT_L(c) ) )[:512],  L = 1024

which is exact up to aliasing of exp(C)'s tail beyond degree L (≈1e-6 here,
validated offline against the recurrence in float64).

On-device pipeline per 512-row supertile (all matmuls fp16 in / fp32 psum):
  - PE transpose c tiles -> cT (contraction dim on partitions)
  - fwd:  R/I[bins=512, rows=512] = Wcos/Wsin-stationary matmuls over cT
  - ACT:  A = exp(R), Sn = sin(I), Cs = sin(I + pi/2)      (bins 1..512)
  - DVE:  HRe = A*Cs, HIm = A*Sn
  - inv:  h[rows, n] = sum_bins HRe*ci + HIm*si  (H-stationary matmuls)
  - DC bin: h += exp(rowsum(c))/L  (vector reduce + ACT exp + broadcast add)

Pure data parallel across 8 cores on the batch axis (131072 = 8 * 16384).
"""

import os
import sys
from contextlib import ExitStack

import numpy as np

for _p in ("/opt/trn_rl_repo", "/root/.axon_site/_ro/trn_rl_repo"):
    if os.path.isdir(_p) and _p not in sys.path:
        sys.path.insert(0, _p)

from concourse import bacc, mybir, tile  # noqa: E402
from concourse.bass_utils import run_bass_kernel_spmd  # noqa: E402

B_TOTAL = 131072
N_CORES = 8
B_CORE = B_TOTAL // N_CORES  # 16384
M1 = 256          # cepstral coefficients per row (M+1)
N_OUT = 512       # impulse response length
L = 1024          # DFT length
NB = L // 2       # matmul-handled bins 1..512 (bin 0 handled via row-sum)
ST_ROWS = 512     # rows per supertile
N_ST = B_CORE // ST_ROWS  # 32

F32 = mybir.dt.float32
F16 = mybir.dt.float16

_cache: dict = {}


def _host_weights():
    d = np.arange(M1, dtype=np.float64)
    k = np.arange(1, NB + 1, dtype=np.float64)
    th = 2.0 * np.pi * np.outer(d, k) / L           # (256, 512)
    wc = np.cos(th)
    ws = -np.sin(th)
    n = np.arange(N_OUT, dtype=np.float64)
    thi = 2.0 * np.pi * np.outer(k, n) / L          # (512, 512)
    w = np.where(k == NB, 1.0, 2.0)[:, None] / L
    ci = w * np.cos(thi)
    si = -w * np.sin(thi)
    ident = np.eye(128)
    return (
        wc.astype(np.float16),
        ws.astype(np.float16),
        ci.astype(np.float16),
        si.astype(np.float16),
        ident.astype(np.float16),
    )


def _build(n_st=N_ST):
    nc = bacc.Bacc(
        "TRN2", target_bir_lowering=False, debug=False, num_devices=N_CORES
    )
    c_ap = nc.dram_tensor("c", [n_st * ST_ROWS, M1], F32, kind="ExternalInput").ap()
    wc_ap = nc.dram_tensor("wcf", [M1, NB], F16, kind="ExternalInput").ap()
    ws_ap = nc.dram_tensor("wsf", [M1, NB], F16, kind="ExternalInput").ap()
    ci_ap = nc.dram_tensor("cif", [NB, N_OUT], F16, kind="ExternalInput").ap()
    si_ap = nc.dram_tensor("sif", [NB, N_OUT], F16, kind="ExternalInput").ap()
    id_ap = nc.dram_tensor("identf", [128, 128], F16, kind="ExternalInput").ap()
    h_ap = nc.dram_tensor("h", [n_st * ST_ROWS, N_OUT], F32, kind="ExternalOutput").ap()

    EXP = mybir.ActivationFunctionType.Exp
    SIN = mybir.ActivationFunctionType.Sin

    with tile.TileContext(nc) as tc, ExitStack() as ctx:
        const = ctx.enter_context(tc.tile_pool(name="const", bufs=1))
        iop = ctx.enter_context(tc.tile_pool(name="iop", bufs=8))
        castp = ctx.enter_context(tc.tile_pool(name="castp", bufs=8))
        ctp = ctx.enter_context(tc.tile_pool(name="ctp", bufs=4))
        actp = ctx.enter_context(tc.tile_pool(name="actp", bufs=2))
        hp = ctx.enter_context(tc.tile_pool(name="hp", bufs=8))
        outp = ctx.enter_context(tc.tile_pool(name="outp", bufs=4))
        dcp = ctx.enter_context(tc.tile_pool(name="dcp", bufs=2))
        ps_t = ctx.enter_context(tc.tile_pool(name="ps_t", bufs=2, space="PSUM"))
        ps_ri = ctx.enter_context(tc.tile_pool(name="ps_ri", bufs=2, space="PSUM"))
        ps_h = ctx.enter_context(tc.tile_pool(name="ps_h", bufs=2, space="PSUM"))

        # constants
        wc_sb = [const.tile([128, NB], F16, tag=f"wc{d}", name=f"wc{d}") for d in range(2)]
        ws_sb = [const.tile([128, NB], F16, tag=f"ws{d}", name=f"ws{d}") for d in range(2)]
        for d in range(2):
            nc.sync.dma_start(wc_sb[d][:], wc_ap[d * 128:(d + 1) * 128, :])
            nc.sync.dma_start(ws_sb[d][:], ws_ap[d * 128:(d + 1) * 128, :])
        ci_sb = [const.tile([128, N_OUT], F16, tag=f"ci{b}", name=f"ci{b}") for b in range(4)]
        si_sb = [const.tile([128, N_OUT], F16, tag=f"si{b}", name=f"si{b}") for b in range(4)]
        for b in range(4):
            nc.sync.dma_start(ci_sb[b][:], ci_ap[b * 128:(b + 1) * 128, :])
            nc.sync.dma_start(si_sb[b][:], si_ap[b * 128:(b + 1) * 128, :])
        ident = const.tile([128, 128], F16, tag="ident")
        nc.sync.dma_start(ident[:], id_ap[:])
        zb = const.tile([128, 1], F32, tag="zb")
        nc.gpsimd.memset(zb[:], 0.0)
        hpi = const.tile([128, 1], F32, tag="hpi")
        nc.gpsimd.memset(hpi[:], float(np.pi / 2))
        negln = const.tile([128, 1], F32, tag="negln")
        nc.gpsimd.memset(negln[:], float(-np.log(L)))

        for st in range(n_st):
            r0 = st * ST_ROWS
            # ---- load + cast + DC row-sums ----
            c_sb = []
            c16 = []
            s0 = dcp.tile([128, 4], F32, tag="s0")
            for rt in range(4):
                t = iop.tile([128, M1], F32, tag="c_sb", name="c_sb")
                nc.sync.dma_start(t[:], c_ap[r0 + rt * 128: r0 + (rt + 1) * 128, :])
                c_sb.append(t)
                t16 = castp.tile([128, M1], F16, tag="c16", name="c16")
                nc.vector.tensor_copy(t16[:], t[:])
                c16.append(t16)
                nc.vector.tensor_reduce(
                    s0[:, rt:rt + 1], t[:], axis=mybir.AxisListType.X,
                    op=mybir.AluOpType.add,
                )
            h0n = dcp.tile([128, 4], F32, tag="h0n")
            nc.scalar.activation(h0n[:], s0[:], EXP, bias=negln[:])

            # ---- transpose c: (rows, d) -> cT chunks (d, rows) ----
            cT_ps = [ps_t.tile([128, ST_ROWS], F16, tag="cT_ps", name="cT_ps") for _ in range(2)]
            for rt in range(4):
                for d in range(2):
                    nc.tensor.transpose(
                        cT_ps[d][:, rt * 128:(rt + 1) * 128],
                        c16[rt][:, d * 128:(d + 1) * 128],
                        ident[:],
                    )
            cT16 = []
            for d in range(2):
                t = ctp.tile([128, ST_ROWS], F16, tag="cT16", name="cT16")
                nc.vector.tensor_copy(t[:], cT_ps[d][:])
                cT16.append(t)

            # ---- forward DFT + pointwise, per bin-chunk ----
            HRe = []
            HIm = []
            for bc in range(4):
                r_ps = ps_ri.tile([128, ST_ROWS], F32, tag="R")
                i_ps = ps_ri.tile([128, ST_ROWS], F32, tag="I")
                for d in range(2):
                    nc.tensor.matmul(
                        r_ps[:], wc_sb[d][:, bc * 128:(bc + 1) * 128], cT16[d][:],
                        start=(d == 0), stop=(d == 1),
                    )
                for d in range(2):
                    nc.tensor.matmul(
                        i_ps[:], ws_sb[d][:, bc * 128:(bc + 1) * 128], cT16[d][:],
                        start=(d == 0), stop=(d == 1),
                    )
                a_sb = actp.tile([128, ST_ROWS], F16, tag="A")
                sn_sb = actp.tile([128, ST_ROWS], F16, tag="Sn")
                cs_sb = actp.tile([128, ST_ROWS], F16, tag="Cs")
                nc.scalar.activation(a_sb[:], r_ps[:], EXP, bias=zb[:])
                nc.scalar.activation(sn_sb[:], i_ps[:], SIN, bias=zb[:])
                nc.scalar.activation(cs_sb[:], i_ps[:], SIN, bias=hpi[:])
                hre = hp.tile([128, ST_ROWS], F16, tag="HRe")
                him = hp.tile([128, ST_ROWS], F16, tag="HIm")
                nc.vector.tensor_mul(hre[:], a_sb[:], cs_sb[:])
                nc.vector.tensor_mul(him[:], a_sb[:], sn_sb[:])
                HRe.append(hre)
                HIm.append(him)

            # ---- inverse DFT per row-chunk + DC add + store ----
            for rc in range(4):
                h_ps = ps_h.tile([128, N_OUT], F32, tag="h_ps")
                for bc in range(4):
                    nc.tensor.matmul(
                        h_ps[:], HRe[bc][:, rc * 128:(rc + 1) * 128], ci_sb[bc][:],
                        start=(bc == 0), stop=False,
                    )
                    nc.tensor.matmul(
                        h_ps[:], HIm[bc][:, rc * 128:(rc + 1) * 128], si_sb[bc][:],
                        start=False, stop=(bc == 3),
                    )
                o_sb = outp.tile([128, N_OUT], F32, tag="o_sb")
                nc.vector.tensor_scalar_add(o_sb[:], h_ps[:], h0n[:, rc:rc + 1])
                nc.sync.dma_start(
                    h_ap[r0 + rc * 128: r0 + (rc + 1) * 128, :], o_sb[:]
                )

    nc.compile()
    return nc


def _get_nc(n_st=N_ST):
    key = ("nc", n_st)
    if key not in _cache:
        _cache[key] = _build(n_st)
    return _cache[key]


def _in_maps(c):
    wc, ws, ci, si, ident = _host_weights()
    return [
        {
            "c": np.ascontiguousarray(c[i * B_CORE:(i + 1) * B_CORE]),
            "wcf": wc, "wsf": ws, "cif": ci, "sif": si, "identf": ident,
        }
        for i in range(N_CORES)
    ]


def kernel(c):
    c = np.ascontiguousarray(np.asarray(c), dtype=np.float32)
    assert c.shape == (B_TOTAL, M1), c.shape
    nc = _get_nc()
    res = run_bass_kernel_spmd(nc, _in_maps(c), list(range(N_CORES)))
    return np.concatenate(
        [res.results[i]["h"] for i in range(N_CORES)], axis=0
    )


# revision 10
# speedup vs baseline: 1.2587x; 1.2587x over previous
"""Cepstrum -> minimum-phase impulse response on 8 Trainium2 NeuronCores.

Math: the reference recurrence  n*h_n = sum_k (k c_k) h_{n-k}, h_0 = exp(c_0)
is exactly the power-series exponential h = exp(C(z)) mod z^512 for the
degree-255 polynomial C. We evaluate it spectrally:

    h = IDFT_L( exp( DFT_L(c) ) )[:512],  L = 1024

which is exact up to aliasing of exp(C)'s tail beyond degree L (≈1e-6 here,
validated offline against the recurrence in float64).

On-device pipeline per 512-row supertile (all matmuls fp16 in / fp32 psum):
  - PE transpose c tiles -> cT (contraction dim on partitions)
  - fwd:  R/I[bins=512, rows=512] = Wcos/Wsin-stationary matmuls over cT
  - ACT:  A = exp(R), Sn = sin(I), Cs = sin(I + pi/2)      (bins 1..512)
  - DVE:  HRe = A*Cs, HIm = A*Sn
  - inv:  h[rows, n] = sum_bins HRe*ci + HIm*si  (H-stationary matmuls)
  - DC bin: h += exp(rowsum(c))/L  (vector reduce + ACT exp + broadcast add)

Pure data parallel across 8 cores on the batch axis (131072 = 8 * 16384).
"""

import os
import sys
from contextlib import ExitStack

import numpy as np

for _p in ("/opt/trn_rl_repo", "/root/.axon_site/_ro/trn_rl_repo"):
    if os.path.isdir(_p) and _p not in sys.path:
        sys.path.insert(0, _p)

from concourse import bacc, mybir, tile  # noqa: E402
from concourse.bass_utils import run_bass_kernel_spmd  # noqa: E402

B_TOTAL = 131072
N_CORES = 8
B_CORE = B_TOTAL // N_CORES  # 16384
M1 = 256          # cepstral coefficients per row (M+1)
N_OUT = 512       # impulse response length
L = 768           # DFT length (aliasing ~2.4e-5 abs, validated offline)
NB = L // 2       # matmul-handled bins 1..NB (bin 0 handled via row-sum)
NBC = NB // 128   # bin chunks
ST_ROWS = 512     # rows per supertile
N_ST = B_CORE // ST_ROWS  # 32

F32 = mybir.dt.float32
F16 = mybir.dt.float16

_cache: dict = {}

TWO_PI = 2.0 * np.pi


def _install_sin2pi_patches():
    """Keep all activations in ONE ACT table set (exp_and_friends = {exp,
    sin2pi}) to avoid per-supertile table reloads (~2.7us each).

    1. Patch bacc's activation-table map so Exp and Sin both resolve to
       exp_and_friends -> bacc emits a single LoadActFuncSet.
    2. Rewrite "Sin" -> "sin2pi" in the BIR json just before walrus; the
       kernel emits Sin with scale=1/(2*pi) so the arguments are already
       in sin2pi's convention (sin2pi(x) = sin(2*pi*x)).
    """
    if _cache.get("patched"):
        return
    import concourse.bacc as _bacc
    import concourse.bass2jax as _b2j

    SIN = mybir.ActivationFunctionType.Sin
    EXP = mybir.ActivationFunctionType.Exp
    _orig_tables = _bacc.get_activation_tables

    def tables_patched(arch):
        t = {k: set(v) for k, v in _orig_tables(arch).items()}
        for k in t:
            t[k].discard(SIN)
            if k != "exp_and_friends":
                t[k].discard(EXP)
        if "exp_and_friends" in t:
            t["exp_and_friends"] |= {SIN, EXP}
        return t

    _bacc.get_activation_tables = tables_patched

    _orig_compile = _b2j.compile_bir_kernel

    def compile_patched(bir_json, *a, **kw):
        if isinstance(bir_json, bytes):
            bir_json = bir_json.replace(b'"func":"Sin"', b'"func":"Sin2pi"')
        else:
            bir_json = bir_json.replace('"func":"Sin"', '"func":"Sin2pi"')
        return _orig_compile(bir_json, *a, **kw)

    _b2j.compile_bir_kernel = compile_patched
    _cache["patched"] = True


def _host_weights():
    d = np.arange(M1, dtype=np.float64)
    k = np.arange(1, NB + 1, dtype=np.float64)
    th = 2.0 * np.pi * np.outer(d, k) / L           # (256, 512)
    wc = np.cos(th)
    ws = -np.sin(th)
    n = np.arange(N_OUT, dtype=np.float64)
    thi = 2.0 * np.pi * np.outer(k, n) / L          # (512, 512)
    w = np.where(k == NB, 1.0, 2.0)[:, None] / L
    ci = w * np.cos(thi)
    si = -w * np.sin(thi)
    ident = np.eye(128)
    ci16 = ci.astype(np.float16)
    si16 = si.astype(np.float16)
    # Coherent part of the fp16 rounding error of ci: H ~= 1 background does
    # not cancel it (validated offline). Subtract the column sums on-device.
    corr = -(ci16.astype(np.float64) - ci).sum(0)          # (512,)
    corr128 = np.broadcast_to(corr.astype(np.float32), (128, N_OUT)).copy()
    return (
        wc.astype(np.float16),
        ws.astype(np.float16),
        ci16,
        si16,
        ident.astype(np.float16),
        corr128,
    )


def _build(n_st=N_ST, repeat=1):
    _install_sin2pi_patches()
    nc = bacc.Bacc(
        "TRN2", target_bir_lowering=False, debug=False, num_devices=N_CORES
    )
    c_ap = nc.dram_tensor("c", [n_st * ST_ROWS, M1], F32, kind="ExternalInput").ap()
    wc_ap = nc.dram_tensor("wcf", [M1, NB], F16, kind="ExternalInput").ap()
    ws_ap = nc.dram_tensor("wsf", [M1, NB], F16, kind="ExternalInput").ap()
    ci_ap = nc.dram_tensor("cif", [NB, N_OUT], F16, kind="ExternalInput").ap()
    si_ap = nc.dram_tensor("sif", [NB, N_OUT], F16, kind="ExternalInput").ap()
    id_ap = nc.dram_tensor("identf", [128, 128], F16, kind="ExternalInput").ap()
    corr_ap = nc.dram_tensor("corrf", [128, N_OUT], F32, kind="ExternalInput").ap()
    h_ap = nc.dram_tensor("h", [n_st * ST_ROWS, N_OUT], F32, kind="ExternalOutput").ap()

    EXP = mybir.ActivationFunctionType.Exp
    SIN = mybir.ActivationFunctionType.Sin

    with tile.TileContext(nc) as tc, ExitStack() as ctx:
        const = ctx.enter_context(tc.tile_pool(name="const", bufs=1))
        iop = ctx.enter_context(tc.tile_pool(name="iop", bufs=8))
        castp = ctx.enter_context(tc.tile_pool(name="castp", bufs=8))
        ctp = ctx.enter_context(tc.tile_pool(name="ctp", bufs=4))
        actp = ctx.enter_context(tc.tile_pool(name="actp", bufs=2))
        hp = ctx.enter_context(tc.tile_pool(name="hp", bufs=8))
        outp = ctx.enter_context(tc.tile_pool(name="outp", bufs=4))
        dcp = ctx.enter_context(tc.tile_pool(name="dcp", bufs=2))
        ps_t = ctx.enter_context(tc.tile_pool(name="ps_t", bufs=2, space="PSUM"))
        ps_ri = ctx.enter_context(tc.tile_pool(name="ps_ri", bufs=2, space="PSUM"))
        ps_h = ctx.enter_context(tc.tile_pool(name="ps_h", bufs=2, space="PSUM"))

        # constants
        wc_sb = [const.tile([128, NB], F16, tag=f"wc{d}", name=f"wc{d}") for d in range(2)]
        ws_sb = [const.tile([128, NB], F16, tag=f"ws{d}", name=f"ws{d}") for d in range(2)]
        for d in range(2):
            nc.sync.dma_start(wc_sb[d][:], wc_ap[d * 128:(d + 1) * 128, :])
            nc.sync.dma_start(ws_sb[d][:], ws_ap[d * 128:(d + 1) * 128, :])
        ci_sb = [const.tile([128, N_OUT], F16, tag=f"ci{b}", name=f"ci{b}") for b in range(NBC)]
        si_sb = [const.tile([128, N_OUT], F16, tag=f"si{b}", name=f"si{b}") for b in range(NBC)]
        for b in range(NBC):
            nc.sync.dma_start(ci_sb[b][:], ci_ap[b * 128:(b + 1) * 128, :])
            nc.sync.dma_start(si_sb[b][:], si_ap[b * 128:(b + 1) * 128, :])
        ident = const.tile([128, 128], F16, tag="ident")
        nc.sync.dma_start(ident[:], id_ap[:])
        corr_sb = const.tile([128, N_OUT], F32, tag="corr_sb")
        nc.sync.dma_start(corr_sb[:], corr_ap[:])
        zb = const.tile([128, 1], F32, tag="zb")
        nc.gpsimd.memset(zb[:], 0.0)
        quarter = const.tile([128, 1], F32, tag="quarter")
        nc.gpsimd.memset(quarter[:], 0.25)
        negln = const.tile([128, 1], F32, tag="negln")
        nc.gpsimd.memset(negln[:], float(-np.log(L)))

        for st in range(n_st * repeat):
            st = st % n_st
            r0 = st * ST_ROWS
            # ---- load + cast + DC row-sums ----
            c_sb = []
            c16 = []
            s0 = dcp.tile([128, 4], F32, tag="s0")
            for rt in range(4):
                t = iop.tile([128, M1], F32, tag="c_sb", name="c_sb")
                nc.sync.dma_start(t[:], c_ap[r0 + rt * 128: r0 + (rt + 1) * 128, :])
                c_sb.append(t)
                t16 = castp.tile([128, M1], F16, tag="c16", name="c16")
                nc.vector.tensor_copy(t16[:], t[:])
                c16.append(t16)
                nc.vector.tensor_reduce(
                    s0[:, rt:rt + 1], t[:], axis=mybir.AxisListType.X,
                    op=mybir.AluOpType.add,
                )
            h0n = dcp.tile([128, 4], F32, tag="h0n")
            nc.scalar.activation(h0n[:], s0[:], EXP, bias=negln[:])

            # ---- transpose c: (rows, d) -> cT chunks (d, rows) ----
            cT_ps = [ps_t.tile([128, ST_ROWS], F16, tag="cT_ps", name="cT_ps") for _ in range(2)]
            for rt in range(4):
                for d in range(2):
                    nc.tensor.transpose(
                        cT_ps[d][:, rt * 128:(rt + 1) * 128],
                        c16[rt][:, d * 128:(d + 1) * 128],
                        ident[:],
                    )
            cT16 = []
            for d in range(2):
                t = ctp.tile([128, ST_ROWS], F16, tag="cT16", name="cT16")
                nc.vector.tensor_copy(t[:], cT_ps[d][:])
                cT16.append(t)

            # ---- forward DFT + pointwise, per bin-chunk ----
            HRe = []
            HIm = []
            for bc in range(NBC):
                r_ps = ps_ri.tile([128, ST_ROWS], F32, tag="R")
                i_ps = ps_ri.tile([128, ST_ROWS], F32, tag="I")
                for d in range(2):
                    nc.tensor.matmul(
                        r_ps[:], wc_sb[d][:, bc * 128:(bc + 1) * 128], cT16[d][:],
                        start=(d == 0), stop=(d == 1),
                    )
                for d in range(2):
                    nc.tensor.matmul(
                        i_ps[:], ws_sb[d][:, bc * 128:(bc + 1) * 128], cT16[d][:],
                        start=(d == 0), stop=(d == 1),
                    )
                a_sb = actp.tile([128, ST_ROWS], F16, tag="A")
                sn_sb = actp.tile([128, ST_ROWS], F16, tag="Sn")
                cs_sb = actp.tile([128, ST_ROWS], F16, tag="Cs")
                nc.scalar.activation(a_sb[:], r_ps[:], EXP, bias=zb[:])
                nc.scalar.activation(sn_sb[:], i_ps[:], SIN, bias=zb[:],
                                     scale=float(1.0 / TWO_PI))
                nc.scalar.activation(cs_sb[:], i_ps[:], SIN, bias=quarter[:],
                                     scale=float(1.0 / TWO_PI))
                hre = hp.tile([128, ST_ROWS], F16, tag="HRe")
                him = hp.tile([128, ST_ROWS], F16, tag="HIm")
                nc.vector.tensor_mul(hre[:], a_sb[:], cs_sb[:])
                nc.vector.tensor_mul(him[:], a_sb[:], sn_sb[:])
                HRe.append(hre)
                HIm.append(him)

            # ---- inverse DFT per row-chunk + DC add + store ----
            for rc in range(4):
                h_ps = ps_h.tile([128, N_OUT], F32, tag="h_ps")
                for bc in range(NBC):
                    nc.tensor.matmul(
                        h_ps[:], HRe[bc][:, rc * 128:(rc + 1) * 128], ci_sb[bc][:],
                        start=(bc == 0), stop=False,
                    )
                    nc.tensor.matmul(
                        h_ps[:], HIm[bc][:, rc * 128:(rc + 1) * 128], si_sb[bc][:],
                        start=False, stop=(bc == NBC - 1),
                    )
                o_sb = outp.tile([128, N_OUT], F32, tag="o_sb")
                nc.vector.scalar_tensor_tensor(
                    o_sb[:], h_ps[:], h0n[:, rc:rc + 1], corr_sb[:],
                    op0=mybir.AluOpType.add, op1=mybir.AluOpType.add,
                )
                nc.sync.dma_start(
                    h_ap[r0 + rc * 128: r0 + (rc + 1) * 128, :], o_sb[:]
                )

    nc.compile()
    return nc


def _get_nc(n_st=N_ST):
    key = ("nc", n_st)
    if key not in _cache:
        _cache[key] = _build(n_st)
    return _cache[key]


def _in_maps(c):
    wc, ws, ci, si, ident, corr = _host_weights()
    return [
        {
            "c": np.ascontiguousarray(c[i * B_CORE:(i + 1) * B_CORE]),
            "wcf": wc, "wsf": ws, "cif": ci, "sif": si, "identf": ident,
            "corrf": corr,
        }
        for i in range(N_CORES)
    ]


def kernel(c):
    c = np.ascontiguousarray(np.asarray(c), dtype=np.float32)
    assert c.shape == (B_TOTAL, M1), c.shape
    nc = _get_nc()
    res = run_bass_kernel_spmd(nc, _in_maps(c), list(range(N_CORES)))
    return np.concatenate(
        [res.results[i]["h"] for i in range(N_CORES)], axis=0
    )
